# revision 48
# baseline (speedup 1.0000x reference)
"""Multi-head attention (B=4, S=2048, D=768, H=12) on 8 TRN2 NeuronCores.

Sharding: core = (batch b, query-half). Each core computes Q for its 1024
query rows and full-sequence K/V for its batch (K/V projection duplicated
across the 2 cores sharing a batch -> zero collectives), then SDPA + o_proj
for its rows. Output rows are disjoint across cores.

Host-side prep (not on the HW critical path): hidden states pre-transposed
to [768, S] bf16, rope cos/sin tables precomputed (sign pattern folded into
the sin table), V bias pre-broadcast to [128, 768]. This removes all PE
transposes, the device rope-table build, and the K=1 bias matmuls.

Device pipeline: attention begins as soon as QT/KT chunk 0 are projected
and roped (~25us). The remaining K/Q chunks and all V tiles are interleaved
into the attention-phase PE stream (sharing the scores psum ring), so the
ScalarE exp stream - the true pacer at ~207us total - starts early and
never waits on a separate projection phase. All psum evictions run on
VectorE (bias fused via per-partition scalar operand); ScalarE does exp
only. o_proj emits row-major [sq, 768] f32 at the end.
"""

from contextlib import ExitStack

import numpy as np

import concourse.bass as bass
import concourse.bacc as bacc
import concourse.mybir as mybir
import concourse.tile as tile
from concourse.bass import ds, ts
from concourse.bass_utils import run_bass_kernel_spmd

F32 = mybir.dt.float32
BF16 = mybir.dt.bfloat16
AF = mybir.ActivationFunctionType

B, S, D, H = 4, 2048, 768, 12
HD = 64
SQ = 1024          # query rows per core
DC = D // 128      # 6 d-chunks (also 6 head pairs)
ST = S // 128      # 16 seq tiles of 128
ROPE_BASE = 10000.0
N_CORES = 8
LAG = 2


def build_nc():
    nc = bacc.Bacc("TRN2", target_bir_lowering=False, debug=False,
                   num_devices=N_CORES)

    # key columns host-permuted so this core's own query half comes first
    # (attention is permutation-invariant over keys; cos/sin/V follow the
    # same permutation) - kills the duplicate q-half load and lets Q proj,
    # K chunk 0 and attention start while the second half streams in.
    # hsT/weights host-packed chunk-major into [128, ...] so each loads in
    # ONE big DMA (per-chunk DMAs arrive ~3us apart per queue and paced the
    # whole preamble).
    hsP = nc.dram_tensor("hsP", [128, DC * S], BF16, kind="ExternalInput")
    cosK = nc.dram_tensor("cosK", [128, S], BF16, kind="ExternalInput")
    sinK = nc.dram_tensor("sinK", [128, S], BF16, kind="ExternalInput")
    wqT = nc.dram_tensor("wqT", [128, DC * D], BF16, kind="ExternalInput")
    wkT = nc.dram_tensor("wkT", [128, DC * D], BF16, kind="ExternalInput")
    wvT = nc.dram_tensor("wvT", [128, DC * D], BF16, kind="ExternalInput")
    woT = nc.dram_tensor("woT", [128, DC * D], BF16, kind="ExternalInput")
    bq = nc.dram_tensor("bq", [128, D // 128], F32, kind="ExternalInput")
    bk = nc.dram_tensor("bk", [128, D // 128], F32, kind="ExternalInput")
    bvB = nc.dram_tensor("bvB", [128, D], BF16, kind="ExternalInput")
    out = nc.dram_tensor("out", [SQ, D], F32, kind="ExternalOutput")

    # rotate_half permutation: out[m] = in[sigma^-1(m)], sigma swaps 32-row
    # blocks 0<->32 and 64<->96 (as matmul lhsT: P[k, sigma(k)] = 1)
    import ml_dtypes
    p_np = np.zeros((128, 128), dtype=np.float32)
    for k in range(128):
        p_np[k, k + 32 if (k // 32) % 2 == 0 else k - 32] = 1.0
    pshift_dram = nc.inline_tensor(p_np.astype(ml_dtypes.bfloat16),
                                   name="pshift")

    with tile.TileContext(nc) as tc:
        _body(nc, tc, hsP, cosK, sinK,
              wqT, wkT, wvT, woT, bq, bk, bvB, out, pshift_dram)
    nc.compile()
    return nc


def _body(nc, tc, hsP, cosK, sinK,
          wqT, wkT, wvT, woT, bq, bk, bvB, out, pshift_dram):
  with ExitStack() as ctx:
    const = ctx.enter_context(tc.tile_pool(name="const", bufs=1))
    persist = ctx.enter_context(tc.tile_pool(name="persist", bufs=1))

    # ---- DMA loads: one big DMA per tensor, spread over the three
    # DMA-capable queues (sync/gpsimd/scalar) ----
    xTbig = persist.tile([128, DC * S], BF16, tag="xT", name="xTbig")
    xT3 = xTbig.rearrange("p (dc c) -> p dc c", c=S)
    nc.sync.dma_start(
        xT3[:, :, 0:SQ],
        hsP[:, 0:DC * SQ].rearrange("p (dc c) -> p dc c", c=SQ))
    xT = [xTbig[:, dc * S:(dc + 1) * S] for dc in range(DC)]
    xq = [xT[dc][:, 0:SQ] for dc in range(DC)]

    pshift = const.tile([128, 128], BF16, tag="pshift", name="pshift_sb")
    nc.gpsimd.dma_start(pshift[:], pshift_dram[:])
    cosK_sb = const.tile([128, S], BF16, tag="cosK", name="cosK_sb")
    sinK_sb = const.tile([128, S], BF16, tag="sinK", name="sinK_sb")
    nc.gpsimd.dma_start(cosK_sb[:, 0:SQ], cosK[:, 0:SQ])
    nc.gpsimd.dma_start(sinK_sb[:, 0:SQ], sinK[:, 0:SQ])
    cosQ_sb = cosK_sb[:, 0:SQ]
    sinQ_sb = sinK_sb[:, 0:SQ]

    def load_weight(wT_dram, name, eng):
        big = persist.tile([128, DC * D], BF16, tag=f"w_{name}",
                           name=f"w_{name}")
        eng.dma_start(big[:], wT_dram[:])
        return [big[:, dc * D:(dc + 1) * D] for dc in range(DC)]

    wq_sb = load_weight(wqT, "q", nc.scalar)
    bq6 = const.tile([128, DC], F32, tag="bq6", name="bq6")
    nc.scalar.dma_start(bq6[:], bq[:])
    bq_sb = [bq6[:, e:e + 1] for e in range(DC)]

    wk_sb = load_weight(wkT, "k", nc.scalar)
    bk6 = const.tile([128, DC], F32, tag="bk6", name="bk6")
    nc.scalar.dma_start(bk6[:], bk[:])
    bk_sb = [bk6[:, e:e + 1] for e in range(DC)]

    wv_sb = load_weight(wvT, "v", nc.gpsimd)
    bvB_sb = const.tile([128, D], BF16, tag="bvB", name="bvB_sb")
    nc.gpsimd.dma_start(bvB_sb[:], bvB[:])

    nc.sync.dma_start(
        xT3[:, :, SQ:S],
        hsP[:, DC * SQ:DC * S].rearrange("p (dc c) -> p dc c", c=SQ))
    nc.gpsimd.dma_start(cosK_sb[:, SQ:S], cosK[:, SQ:S])
    nc.gpsimd.dma_start(sinK_sb[:, SQ:S], sinK[:, SQ:S])

    wo_sb = load_weight(woT, "o", nc.sync)

    # ---- persistent activation tensors ----
    QT = [persist.tile([128, SQ], BF16, tag=f"QT{e}", name=f"QT{e}")
          for e in range(DC)]
    KT = [persist.tile([128, S], BF16, tag=f"KT{e}", name=f"KT{e}")
          for e in range(DC)]
    # width 12*65 + 63: PV lhsT reads a full 128-wide window per head so
    # the PE array runs fully occupied; psum rows 65-127 are junk
    Vaug = [persist.tile([128, H * 65 + 63], BF16, tag=f"Vaug{st}",
                         name=f"Vaug{st}") for st in range(ST)]
    attnT = [persist.tile([128, SQ], BF16, tag=f"attnT{e}", name=f"attnT{e}")
             for e in range(DC)]

    # o_proj partial sums over attnT chunks 0-4, filled during pair 5's
    # slack so the post-attention tail is just the chunk-5 matmul + add
    opart = [persist.tile([128, D], F32, tag=f"opart{st}", name=f"opart{st}")
             for st in range(SQ // 128)]

    # ones column per head + zero tail, set once up front (gpsimd)
    for st in range(ST):
        va = Vaug[st][:, 0:H * 65].rearrange("p (h x) -> p h x", x=65)
        nc.gpsimd.memset(va[:, :, 64:65], 1.0)
        nc.gpsimd.memset(Vaug[st][:, H * 65:], 0.0)

    # ---- pools for the pipelined compute ----
    # psum budget: sc ring 2x[128,1024]=4 banks (also shared by all
    # projection slices), pv 2x[128,1024]=4 banks
    att_ctx = ExitStack()
    scps = att_ctx.enter_context(tc.tile_pool(name="scps", bufs=2,
                                              space="PSUM"))
    pvps = att_ctx.enter_context(tc.tile_pool(name="pvps", bufs=2,
                                              space="PSUM"))
    expp = att_ctx.enter_context(tc.tile_pool(name="expp", bufs=5))
    ropep = att_ctx.enter_context(tc.tile_pool(name="ropep", bufs=2))
    normp = att_ctx.enter_context(tc.tile_pool(name="normp", bufs=1))

    def rope_cols(dst, cosT, sinT, c0, n, kind):
        # dst[:, c0:c0+n] = dst*cos + shift(dst)*sin on a column chunk; sin
        # sign pattern folded into sinT; shift (rotate_half per head = swap
        # 32-row blocks 0<->32, 64<->96) via a PE matmul against the constant
        # permutation matrix (no DMA descgen, ~213ns PE). Chunked so no
        # single DVE op head-of-line-blocks the eviction stream.
        p = scps.tile([128, 512], F32, tag="sc", name="shp")
        nc.tensor.matmul(p[:, 0:n], pshift[:], dst[:, ds(c0, n)],
                         start=True, stop=True)
        tmp = ropep.tile([128, n], BF16, tag="tmp", name="rtmp")
        nc.vector.tensor_mul(tmp[:], p[:, 0:n], sinT[:, ds(c0, n)])
        nc.vector.tensor_mul(dst[:, ds(c0, n)], dst[:, ds(c0, n)],
                             cosT[:, ds(c0, n)])
        nc.vector.tensor_add(dst[:, ds(c0, n)], dst[:, ds(c0, n)], tmp[:])

    def rope(dst, cosT, sinT, n, kind):
        for c0 in range(0, n, 512):
            rope_cols(dst, cosT, sinT, c0, 512, kind)

    def q_slice(e, sl):
        p = scps.tile([128, 512], F32, tag="sc", name="qp")
        for dc in range(DC):
            nc.tensor.matmul(p[:], wq_sb[dc][:, ts(e, 128)],
                             xq[dc][:, ts(sl, 512)],
                             start=(dc == 0), stop=(dc == DC - 1))
        nc.vector.tensor_scalar_add(QT[e][:, ts(sl, 512)], p[:], bq_sb[e][:])

    def q_chunk(e):
        for sl in range(SQ // 512):
            q_slice(e, sl)
        rope(QT[e], cosQ_sb, sinQ_sb, SQ, "q")

    def k_slice(e, sl):
        p = scps.tile([128, 512], F32, tag="sc", name="kp")
        for dc in range(DC):
            nc.tensor.matmul(p[:], wk_sb[dc][:, ts(e, 128)],
                             xT[dc][:, ts(sl, 512)],
                             start=(dc == 0), stop=(dc == DC - 1))
        nc.vector.tensor_scalar_add(KT[e][:, ts(sl, 512)], p[:], bk_sb[e][:])

    def k_chunk(e):
        for sl in range(S // 512):
            k_slice(e, sl)
        rope(KT[e], cosK_sb, sinK_sb, S, "k")

    def v_nt(st, nt):
        p = scps.tile([128, 512], F32, tag="sc", name="vp")
        for dc in range(DC):
            nc.tensor.matmul(p[:, 0:384], xT[dc][:, ts(st, 128)],
                             wv_sb[dc][:, ts(nt, 384)],
                             start=(dc == 0), stop=(dc == DC - 1))
        dst = Vaug[st][:, 0:H * 65].rearrange("p (h x) -> p h x", x=65)
        nc.vector.tensor_add(
            dst[:, ds(nt * 6, 6), 0:64],
            p[:, 0:384].rearrange("p (h hd) -> p h hd", hd=64),
            bvB_sb[:, ts(nt, 384)].rearrange("p (h hd) -> p h hd", hd=64))

    def v_tile(st):
        v_nt(st, 0)
        v_nt(st, 1)

    # ---- preamble: own-half projections for pairs 0-1; the other key half
    # is still streaming in and is absorbed into the early attention steps ----
    q_chunk(0)
    for sl in range(2):
        k_slice(0, sl)
        rope_cols(KT[0], cosK_sb, sinK_sb, 512 * sl, 512, "k")
    q_chunk(1)
    for sl in range(2):
        k_slice(1, sl)
        rope_cols(KT[1], cosK_sb, sinK_sb, 512 * sl, 512, "k")

    # interleave schedule for the attention phase: (hp, skt) -> closures,
    # each item <= ~1.3us PE / ~1us DVE so nothing head-of-line-blocks the
    # psum ring. pair 0 absorbs the other-half K0 slices + V tiles 2..15;
    # pair 1 absorbs other-half K1; pairs 1..4 absorb chunk hp+1.
    sched = {}

    def krope_item(e, sl):
        return lambda: rope_cols(KT[e], cosK_sb, sinK_sb, 512 * sl, 512, "k")

    sched[(0, 2)] = [lambda: k_slice(0, 2)]
    sched[(0, 4)] = [krope_item(0, 2)]
    sched[(0, 5)] = [lambda: k_slice(0, 3)]
    sched[(0, 7)] = [krope_item(0, 3)]
    items0 = [(lambda st=st, nt=nt: v_nt(st, nt))
              for st in range(ST) for nt in range(2)]
    for idx, fn in enumerate(items0):          # 32 V items: 2/step
        sched.setdefault((0, idx // 2), []).append(fn)
    sched.setdefault((1, 0), []).append(lambda: k_slice(1, 2))
    sched.setdefault((1, 1), []).append(lambda: k_slice(1, 3))
    sched.setdefault((1, 2), []).append(krope_item(1, 2))
    sched.setdefault((1, 3), []).append(krope_item(1, 3))
    for hp in range(1, 5):
        e = hp + 1
        for sl in range(4):
            sched.setdefault((hp, 4 + sl), []).append(
                lambda e=e, sl=sl: k_slice(e, sl))
            sched.setdefault((hp, 8 + sl), []).append(krope_item(e, sl))
        for sl in range(2):
            sched.setdefault((hp, 10 + sl), []).append(
                lambda e=e, sl=sl: q_slice(e, sl))
            sched.setdefault((hp, 12 + sl), []).append(
                lambda e=e, sl=sl: rope_cols(QT[e], cosQ_sb, sinQ_sb,
                                             512 * sl, 512, "q"))

    def o_partial(st, nt):
        p = scps.tile([128, 512], F32, tag="sc", name="opp")
        for dc in range(DC - 1):
            nc.tensor.matmul(p[:, 0:384], attnT[dc][:, ts(st, 128)],
                             wo_sb[dc][:, ts(nt, 384)],
                             start=(dc == 0), stop=(dc == DC - 2))
        nc.vector.tensor_copy(opart[st][:, ts(nt, 384)], p[:, 0:384])

    # 10 o-partials inside pair 5's slack; the last 6 run post-loop,
    # overlapping the final pair's normalize transport chain
    o_items = [(st, nt) for st in range(SQ // 128) for nt in range(2)]
    for idx, (st, nt) in enumerate(o_items[:10]):
        sched.setdefault((5, 2 + idx), []).append(
            lambda st=st, nt=nt: o_partial(st, nt))
    o_items_tail = o_items[10:]

    # ---- attention ----
    pending_norm = [None]
    for hp in range(DC):
        pvs = [pvps.tile([128, SQ], F32, tag="pv", name=f"pv{i}")
               for i in range(2)]
        ex = [[None] * ST, [None] * ST]

        def do_pv(j, hp=hp, pvs=pvs, ex=ex):
            for i in range(2):
                h = 2 * hp + i
                for jj in range(SQ // 512):
                    nc.tensor.matmul(
                        pvs[i][:, ts(jj, 512)],
                        Vaug[j][:, ds(h * 65, 128)],
                        ex[i][j][:, ts(jj, 512)],
                        start=(j == 0), stop=(j == ST - 1))

        for skt in range(ST):
            fns = list(sched.get((hp, skt), ()))
            for i in range(2):
                sc = scps.tile([128, SQ], F32, tag="sc", name="sc")
                for j in range(SQ // 512):
                    nc.tensor.matmul(
                        sc[:, ts(j, 512)],
                        KT[hp][ds(64 * i, 64), ts(skt, 128)],
                        QT[hp][ds(64 * i, 64), ts(j, 512)],
                        start=True, stop=True,
                        tile_position=(64 * i, 0))
                e_t = expp.tile([128, SQ], BF16, tag="exp", name="expt")
                nc.scalar.activation(e_t[:], sc[:], AF.Exp, scale=0.125)
                ex[i][skt] = e_t
                if fns:  # interleave one proj item after each head's scores
                    fns.pop(0)()
            if skt == 1 and pending_norm[0] is not None:
                pending_norm[0]()
                pending_norm[0] = None
            if skt >= LAG:
                do_pv(skt - LAG)
            for fn in fns:
                fn()
        for j in range(ST - LAG, ST):
            do_pv(j)

        # normalize: psum row 64 = softmax denominator (ones column in
        # Vaug). Transport (DMA to partition-major, fast [128,8] exact
        # reciprocal, DMA back, broadcast) emitted now; the psum-releasing
        # scale-evict muls are deferred into the next pair's step 1 so the
        # PE stream stays dense across the pair boundary.
        rbs_list = []
        for i in range(2):
            rsrow = normp.tile([1, SQ], F32, tag=f"rsrow_{i}", name="rsrow")
            for c in range(2):
                nc.vector.tensor_copy(rsrow[:, ts(c, 512)],
                                      pvs[i][ds(64, 1), ts(c, 512)])
            c8 = normp.tile([128, SQ // 128], F32, tag=f"c8_{i}", name="c8")
            nc.sync.dma_start(c8[:], rsrow[:])
            r8 = normp.tile([128, SQ // 128], F32, tag=f"r8_{i}", name="r8")
            nc.vector.reciprocal(r8[:], c8[:])
            recb = normp.tile([1, SQ], F32, tag=f"recb_{i}", name="recb")
            nc.sync.dma_start(recb[:], r8[:])
            rbs = normp.tile([64, SQ], F32, tag=f"rbs_{i}", name="rbs")
            nc.gpsimd.partition_broadcast(rbs[:], recb[:])
            rbs_list.append(rbs)

        def norm_pair(hp=hp, pvs=pvs, rbs_list=rbs_list):
            for c in range(2):      # chunked so DVE stays interruptible;
                for i in range(2):  # c-outer so o-tail st 0-3 unblock first
                    nc.vector.tensor_mul(
                        attnT[hp][ds(64 * i, 64), ts(c, 512)],
                        pvs[i][ds(0, 64), ts(c, 512)],
                        rbs_list[i][:, ts(c, 512)])
        pending_norm[0] = norm_pair
    for st, nt in o_items_tail:  # overlap the last pair's norm transport
        o_partial(st, nt)
    pending_norm[0]()
    att_ctx.close()

    # ---- o_proj tail: chunk-5 matmul + add to the pair-5 partials,
    # evict-adds split across ScalarE/VectorE, one [128, 768] DMA per tile ----
    with (tc.tile_pool(name="o_ps", bufs=4, space="PSUM") as ops,
          tc.tile_pool(name="o_sb", bufs=3) as osb):
        for st in range(SQ // 128):
            o = osb.tile([128, D], F32, tag="o_out", name="o_out")
            for nt in range(2):
                p = ops.tile([128, 384], F32, tag="o", name="o_p")
                nc.tensor.matmul(p[:], attnT[DC - 1][:, ts(st, 128)],
                                 wo_sb[DC - 1][:, ts(nt, 384)],
                                 start=True, stop=True)
                nc.vector.tensor_add(o[:, ts(nt, 384)], p[:],
                                     opart[st][:, ts(nt, 384)])
            nc.sync.dma_start(out[ts(st, 128), :], o[:])


_NC_CACHE = None


def _get_nc():
    global _NC_CACHE
    if _NC_CACHE is None:
        _NC_CACHE = build_nc()
    return _NC_CACHE


def _rope_tables(pos):
    # [128, n] cos/sin tables in T-layout: rows = 4 blocks of the 32
    # frequencies (2 heads x concat(freqs, freqs)); sin sign pattern folded
    # (rows 0:32 -> -sin for the -x2 half, rows 32:64 -> +sin, repeating)
    import ml_dtypes
    inv = ROPE_BASE ** (-np.arange(32, dtype=np.float64) / 32.0)
    ang = np.outer(inv, pos.astype(np.float64))  # [32, n]
    c32 = np.cos(ang)
    s32 = np.sin(ang)
    cosR = np.tile(c32, (4, 1)).astype(ml_dtypes.bfloat16)
    sinS = np.concatenate([-s32, s32, -s32, s32], axis=0).astype(
        ml_dtypes.bfloat16)
    return np.ascontiguousarray(cosR), np.ascontiguousarray(sinS)


def _pack_chunks(a):
    # [768, N] -> [128, 6*N] chunk-major (one flat DMA per tensor)
    n = a.shape[1]
    return np.ascontiguousarray(
        a.reshape(DC, 128, n).transpose(1, 0, 2).reshape(128, DC * n))


def kernel(hidden_states, position_ids, wq, bq, wk, bk, wv, bv, wo,
           _trace=False):
    import ml_dtypes
    bf16 = ml_dtypes.bfloat16
    hidden_states = np.asarray(hidden_states, dtype=np.float32)
    position_ids = np.asarray(position_ids, dtype=np.int32)
    wqT = _pack_chunks(np.asarray(wq, np.float32).T.astype(bf16))
    wkT = _pack_chunks(np.asarray(wk, np.float32).T.astype(bf16))
    wvT = _pack_chunks(np.asarray(wv, np.float32).T.astype(bf16))
    woT = _pack_chunks(np.asarray(wo, np.float32).T.astype(bf16))
    bq_c = np.ascontiguousarray(np.asarray(bq, np.float32).reshape(DC, 128).T)
    bk_c = np.ascontiguousarray(np.asarray(bk, np.float32).reshape(DC, 128).T)
    bvB = np.ascontiguousarray(
        np.broadcast_to(np.asarray(bv, np.float32).astype(bf16)[None, :],
                        (128, D)))

    nc = _get_nc()
    in_maps = []
    for b in range(B):
        # key columns permuted per core so its own query half comes first
        hsT_b = hidden_states[b].T.astype(bf16)  # [768, 2048]
        cosK_b, sinK_b = _rope_tables(position_ids[b])
        for half in range(2):
            if half == 0:
                perm = lambda a: np.ascontiguousarray(a)
            else:
                perm = lambda a: np.ascontiguousarray(
                    np.concatenate([a[:, SQ:], a[:, :SQ]], axis=1))
            hs_p = perm(hsT_b).reshape(DC, 128, S)  # [dc, p, c] permuted
            # [own-half pack | other-half pack], each chunk-major
            hs_packed = np.concatenate(
                [hs_p[:, :, 0:SQ].transpose(1, 0, 2).reshape(128, DC * SQ),
                 hs_p[:, :, SQ:S].transpose(1, 0, 2).reshape(128, DC * SQ)],
                axis=1)
            in_maps.append({
                "hsP": np.ascontiguousarray(hs_packed),
                "cosK": perm(cosK_b), "sinK": perm(sinK_b),
                "wqT": wqT, "wkT": wkT, "wvT": wvT, "woT": woT,
                "bq": bq_c, "bk": bk_c, "bvB": bvB,
            })
    res = run_bass_kernel_spmd(nc, in_maps, list(range(N_CORES)),
                               trace=_trace)
    outp = np.empty((B, S, D), np.float32)
    for core in range(N_CORES):
        b, half = core // 2, core % 2
        outp[b, half * SQ:(half + 1) * SQ] = res.results[core]["out"]
    if _trace:
        kernel._last_exec_time_ns = res.exec_time_ns
        kernel._last_results = res
    return outp


# revision 53
# speedup vs baseline: 1.0086x; 1.0086x over previous
"""Multi-head attention (B=4, S=2048, D=768, H=12) on 8 TRN2 NeuronCores.

Sharding: core = (batch b, query-half). Each core computes Q for its 1024
query rows and full-sequence K/V for its batch (K/V projection duplicated
across the 2 cores sharing a batch -> zero collectives), then SDPA + o_proj
for its rows. Output rows are disjoint across cores.

Host-side prep (not on the HW critical path): hidden states pre-transposed
to [768, S] bf16, rope cos/sin tables precomputed (sign pattern folded into
the sin table), V bias pre-broadcast to [128, 768]. This removes all PE
transposes, the device rope-table build, and the K=1 bias matmuls.

Device pipeline: attention begins as soon as QT/KT chunk 0 are projected
and roped (~25us). The remaining K/Q chunks and all V tiles are interleaved
into the attention-phase PE stream (sharing the scores psum ring), so the
ScalarE exp stream - the true pacer at ~207us total - starts early and
never waits on a separate projection phase. All psum evictions run on
VectorE (bias fused via per-partition scalar operand); ScalarE does exp
only. o_proj emits row-major [sq, 768] f32 at the end.
"""

from contextlib import ExitStack

import numpy as np

import concourse.bass as bass
import concourse.bacc as bacc
import concourse.mybir as mybir
import concourse.tile as tile
from concourse.bass import ds, ts
from concourse.bass_utils import run_bass_kernel_spmd

F32 = mybir.dt.float32
BF16 = mybir.dt.bfloat16
AF = mybir.ActivationFunctionType

B, S, D, H = 4, 2048, 768, 12
HD = 64
SQ = 1024          # query rows per core
DC = D // 128      # 6 d-chunks (also 6 head pairs)
ST = S // 128      # 16 seq tiles of 128
ROPE_BASE = 10000.0
N_CORES = 8
LAG = 2


def build_nc():
    nc = bacc.Bacc("TRN2", target_bir_lowering=False, debug=False,
                   num_devices=N_CORES)

    # key columns host-permuted so this core's own query half comes first
    # (attention is permutation-invariant over keys; cos/sin/V follow the
    # same permutation) - kills the duplicate q-half load and lets Q proj,
    # K chunk 0 and attention start while the second half streams in.
    # hsT/weights host-packed chunk-major into [128, ...] so each loads in
    # ONE big DMA (per-chunk DMAs arrive ~3us apart per queue and paced the
    # whole preamble).
    hsP = nc.dram_tensor("hsP", [128, DC * S], BF16, kind="ExternalInput")
    cosK = nc.dram_tensor("cosK", [128, S], BF16, kind="ExternalInput")
    sinK = nc.dram_tensor("sinK", [128, S], BF16, kind="ExternalInput")
    wqT = nc.dram_tensor("wqT", [128, DC * D], BF16, kind="ExternalInput")
    wkT = nc.dram_tensor("wkT", [128, DC * D], BF16, kind="ExternalInput")
    wvT = nc.dram_tensor("wvT", [128, DC * D], BF16, kind="ExternalInput")
    woT = nc.dram_tensor("woT", [128, DC * D], BF16, kind="ExternalInput")
    bq = nc.dram_tensor("bq", [128, D // 128], F32, kind="ExternalInput")
    bk = nc.dram_tensor("bk", [128, D // 128], F32, kind="ExternalInput")
    bvB = nc.dram_tensor("bvB", [128, D], BF16, kind="ExternalInput")
    out = nc.dram_tensor("out", [SQ, D], F32, kind="ExternalOutput")

    # rotate_half permutation: out[m] = in[sigma^-1(m)], sigma swaps 32-row
    # blocks 0<->32 and 64<->96 (as matmul lhsT: P[k, sigma(k)] = 1)
    import ml_dtypes
    p_np = np.zeros((128, 128), dtype=np.float32)
    for k in range(128):
        p_np[k, k + 32 if (k // 32) % 2 == 0 else k - 32] = 1.0
    pshift_dram = nc.inline_tensor(p_np.astype(ml_dtypes.bfloat16),
                                   name="pshift")

    with tile.TileContext(nc) as tc:
        _body(nc, tc, hsP, cosK, sinK,
              wqT, wkT, wvT, woT, bq, bk, bvB, out, pshift_dram)
    nc.compile()
    return nc


def _body(nc, tc, hsP, cosK, sinK,
          wqT, wkT, wvT, woT, bq, bk, bvB, out, pshift_dram):
  with ExitStack() as ctx:
    const = ctx.enter_context(tc.tile_pool(name="const", bufs=1))
    persist = ctx.enter_context(tc.tile_pool(name="persist", bufs=1))

    # ---- DMA loads: one big DMA per tensor, spread over the three
    # DMA-capable queues (sync/gpsimd/scalar) ----
    xTbig = persist.tile([128, DC * S], BF16, tag="xT", name="xTbig")
    xT3 = xTbig.rearrange("p (dc c) -> p dc c", c=S)
    nc.sync.dma_start(
        xT3[:, :, 0:SQ],
        hsP[:, 0:DC * SQ].rearrange("p (dc c) -> p dc c", c=SQ))
    xT = [xTbig[:, dc * S:(dc + 1) * S] for dc in range(DC)]
    xq = [xT[dc][:, 0:SQ] for dc in range(DC)]

    pshift = const.tile([128, 128], BF16, tag="pshift", name="pshift_sb")
    nc.gpsimd.dma_start(pshift[:], pshift_dram[:])
    cosK_sb = const.tile([128, S], BF16, tag="cosK", name="cosK_sb")
    sinK_sb = const.tile([128, S], BF16, tag="sinK", name="sinK_sb")
    nc.gpsimd.dma_start(cosK_sb[:, 0:SQ], cosK[:, 0:SQ])
    nc.gpsimd.dma_start(sinK_sb[:, 0:SQ], sinK[:, 0:SQ])
    cosQ_sb = cosK_sb[:, 0:SQ]
    sinQ_sb = sinK_sb[:, 0:SQ]

    def load_weight(wT_dram, name, eng):
        big = persist.tile([128, DC * D], BF16, tag=f"w_{name}",
                           name=f"w_{name}")
        eng.dma_start(big[:], wT_dram[:])
        return [big[:, dc * D:(dc + 1) * D] for dc in range(DC)]

    # wq/wk are packed OUTPUT-chunk-major on the host (w2[p, (e, dc, j)] =
    # wT[dc*128+p, e*128+j]) and loaded in two priority DMAs each, so Q/K
    # chunks 0-1 - which gate attention start - only wait for the first
    # third of each weight. wq_sb2(e)[dc] is the [128, 128] lhsT block.
    def load_weight_emajor(wT_dram, name, eng):
        big = persist.tile([128, DC * D], BF16, tag=f"w_{name}",
                           name=f"w_{name}")
        eng.dma_start(big[:, 0:2 * DC * 128], wT_dram[:, 0:2 * DC * 128])
        return big

    wq_big = load_weight_emajor(wqT, "q", nc.scalar)
    bq6 = const.tile([128, DC], F32, tag="bq6", name="bq6")
    nc.scalar.dma_start(bq6[:], bq[:])
    bq_sb = [bq6[:, e:e + 1] for e in range(DC)]

    wk_big = load_weight_emajor(wkT, "k", nc.scalar)
    bk6 = const.tile([128, DC], F32, tag="bk6", name="bk6")
    nc.scalar.dma_start(bk6[:], bk[:])
    bk_sb = [bk6[:, e:e + 1] for e in range(DC)]

    nc.scalar.dma_start(wq_big[:, 2 * DC * 128:], wqT[:, 2 * DC * 128:])
    nc.scalar.dma_start(wk_big[:, 2 * DC * 128:], wkT[:, 2 * DC * 128:])

    def w_emajor(big, e):
        return [big[:, (e * DC + dc) * 128:(e * DC + dc + 1) * 128]
                for dc in range(DC)]

    wv_sb = load_weight(wvT, "v", nc.gpsimd)
    bvB_sb = const.tile([128, D], BF16, tag="bvB", name="bvB_sb")
    nc.gpsimd.dma_start(bvB_sb[:], bvB[:])

    nc.sync.dma_start(
        xT3[:, :, SQ:S],
        hsP[:, DC * SQ:DC * S].rearrange("p (dc c) -> p dc c", c=SQ))
    nc.gpsimd.dma_start(cosK_sb[:, SQ:S], cosK[:, SQ:S])
    nc.gpsimd.dma_start(sinK_sb[:, SQ:S], sinK[:, SQ:S])

    wo_sb = load_weight(woT, "o", nc.sync)

    # ---- persistent activation tensors ----
    QT = [persist.tile([128, SQ], BF16, tag=f"QT{e}", name=f"QT{e}")
          for e in range(DC)]
    KT = [persist.tile([128, S], BF16, tag=f"KT{e}", name=f"KT{e}")
          for e in range(DC)]
    # width 12*65 + 63: PV lhsT reads a full 128-wide window per head so
    # the PE array runs fully occupied; psum rows 65-127 are junk
    Vaug = [persist.tile([128, H * 65 + 63], BF16, tag=f"Vaug{st}",
                         name=f"Vaug{st}") for st in range(ST)]
    attnT = [persist.tile([128, SQ], BF16, tag=f"attnT{e}", name=f"attnT{e}")
             for e in range(DC)]

    # o_proj partial sums over attnT chunks 0-4, filled during pair 5's
    # slack so the post-attention tail is just the chunk-5 matmul + add
    opart = [persist.tile([128, D], F32, tag=f"opart{st}", name=f"opart{st}")
             for st in range(SQ // 128)]

    # ones column per head + zero tail, set once up front (gpsimd)
    for st in range(ST):
        va = Vaug[st][:, 0:H * 65].rearrange("p (h x) -> p h x", x=65)
        nc.gpsimd.memset(va[:, :, 64:65], 1.0)
        nc.gpsimd.memset(Vaug[st][:, H * 65:], 0.0)

    # ---- pools for the pipelined compute ----
    # psum budget: sc ring 2x[128,1024]=4 banks (also shared by all
    # projection slices), pv 2x[128,1024]=4 banks
    att_ctx = ExitStack()
    scps = att_ctx.enter_context(tc.tile_pool(name="scps", bufs=2,
                                              space="PSUM"))
    pvps = att_ctx.enter_context(tc.tile_pool(name="pvps", bufs=2,
                                              space="PSUM"))
    expp = att_ctx.enter_context(tc.tile_pool(name="expp", bufs=5))
    ropep = att_ctx.enter_context(tc.tile_pool(name="ropep", bufs=2))
    normp = att_ctx.enter_context(tc.tile_pool(name="normp", bufs=1))

    def rope_cols(dst, cosT, sinT, c0, n, kind):
        # dst[:, c0:c0+n] = dst*cos + shift(dst)*sin on a column chunk; sin
        # sign pattern folded into sinT; shift (rotate_half per head = swap
        # 32-row blocks 0<->32, 64<->96) via a PE matmul against the constant
        # permutation matrix (no DMA descgen, ~213ns PE). Chunked so no
        # single DVE op head-of-line-blocks the eviction stream.
        p = scps.tile([128, 512], F32, tag="sc", name="shp")
        nc.tensor.matmul(p[:, 0:n], pshift[:], dst[:, ds(c0, n)],
                         start=True, stop=True)
        tmp = ropep.tile([128, n], BF16, tag="tmp", name="rtmp")
        nc.vector.tensor_mul(tmp[:], p[:, 0:n], sinT[:, ds(c0, n)])
        nc.vector.tensor_mul(dst[:, ds(c0, n)], dst[:, ds(c0, n)],
                             cosT[:, ds(c0, n)])
        nc.vector.tensor_add(dst[:, ds(c0, n)], dst[:, ds(c0, n)], tmp[:])

    def rope(dst, cosT, sinT, n, kind):
        for c0 in range(0, n, 512):
            rope_cols(dst, cosT, sinT, c0, 512, kind)

    def q_slice(e, sl):
        p = scps.tile([128, 512], F32, tag="sc", name="qp")
        we = w_emajor(wq_big, e)
        for dc in range(DC):
            nc.tensor.matmul(p[:], we[dc],
                             xq[dc][:, ts(sl, 512)],
                             start=(dc == 0), stop=(dc == DC - 1))
        nc.vector.tensor_scalar_add(QT[e][:, ts(sl, 512)], p[:], bq_sb[e][:])

    def q_chunk(e):
        for sl in range(SQ // 512):
            q_slice(e, sl)
        rope(QT[e], cosQ_sb, sinQ_sb, SQ, "q")

    def k_slice(e, sl):
        p = scps.tile([128, 512], F32, tag="sc", name="kp")
        we = w_emajor(wk_big, e)
        for dc in range(DC):
            nc.tensor.matmul(p[:], we[dc],
                             xT[dc][:, ts(sl, 512)],
                             start=(dc == 0), stop=(dc == DC - 1))
        nc.vector.tensor_scalar_add(KT[e][:, ts(sl, 512)], p[:], bk_sb[e][:])

    def k_chunk(e):
        for sl in range(S // 512):
            k_slice(e, sl)
        rope(KT[e], cosK_sb, sinK_sb, S, "k")

    def v_nt(st, nt):
        p = scps.tile([128, 512], F32, tag="sc", name="vp")
        for dc in range(DC):
            nc.tensor.matmul(p[:, 0:384], xT[dc][:, ts(st, 128)],
                             wv_sb[dc][:, ts(nt, 384)],
                             start=(dc == 0), stop=(dc == DC - 1))
        dst = Vaug[st][:, 0:H * 65].rearrange("p (h x) -> p h x", x=65)
        nc.vector.tensor_add(
            dst[:, ds(nt * 6, 6), 0:64],
            p[:, 0:384].rearrange("p (h hd) -> p h hd", hd=64),
            bvB_sb[:, ts(nt, 384)].rearrange("p (h hd) -> p h hd", hd=64))

    def v_tile(st):
        v_nt(st, 0)
        v_nt(st, 1)

    # ---- preamble: own-half projections for pairs 0-1; the other key half
    # is still streaming in and is absorbed into the early attention steps ----
    q_chunk(0)
    for sl in range(2):
        k_slice(0, sl)
        rope_cols(KT[0], cosK_sb, sinK_sb, 512 * sl, 512, "k")
    q_chunk(1)
    for sl in range(2):
        k_slice(1, sl)
        rope_cols(KT[1], cosK_sb, sinK_sb, 512 * sl, 512, "k")

    # interleave schedule for the attention phase: (hp, skt) -> closures,
    # each item <= ~1.3us PE / ~1us DVE so nothing head-of-line-blocks the
    # psum ring. pair 0 absorbs the other-half K0 slices + V tiles 2..15;
    # pair 1 absorbs other-half K1; pairs 1..4 absorb chunk hp+1.
    sched = {}

    def krope_item(e, sl):
        return lambda: rope_cols(KT[e], cosK_sb, sinK_sb, 512 * sl, 512, "k")

    sched[(0, 2)] = [lambda: k_slice(0, 2)]
    sched[(0, 4)] = [krope_item(0, 2)]
    sched[(0, 5)] = [lambda: k_slice(0, 3)]
    sched[(0, 7)] = [krope_item(0, 3)]
    items0 = [(lambda st=st, nt=nt: v_nt(st, nt))
              for st in range(ST) for nt in range(2)]
    for idx, fn in enumerate(items0):          # 32 V items: 2/step
        sched.setdefault((0, idx // 2), []).append(fn)
    sched.setdefault((1, 0), []).append(lambda: k_slice(1, 2))
    sched.setdefault((1, 1), []).append(lambda: k_slice(1, 3))
    sched.setdefault((1, 2), []).append(krope_item(1, 2))
    sched.setdefault((1, 3), []).append(krope_item(1, 3))
    for hp in range(1, 5):
        e = hp + 1
        for sl in range(4):
            sched.setdefault((hp, 4 + sl), []).append(
                lambda e=e, sl=sl: k_slice(e, sl))
            sched.setdefault((hp, 8 + sl), []).append(krope_item(e, sl))
        for sl in range(2):
            sched.setdefault((hp, 10 + sl), []).append(
                lambda e=e, sl=sl: q_slice(e, sl))
            sched.setdefault((hp, 12 + sl), []).append(
                lambda e=e, sl=sl: rope_cols(QT[e], cosQ_sb, sinQ_sb,
                                             512 * sl, 512, "q"))

    def o_partial(st, nt):
        p = scps.tile([128, 512], F32, tag="sc", name="opp")
        for dc in range(DC - 1):
            nc.tensor.matmul(p[:, 0:384], attnT[dc][:, ts(st, 128)],
                             wo_sb[dc][:, ts(nt, 384)],
                             start=(dc == 0), stop=(dc == DC - 2))
        nc.vector.tensor_copy(opart[st][:, ts(nt, 384)], p[:, 0:384])

    # 10 o-partials inside pair 5's slack; the last 6 run post-loop,
    # overlapping the final pair's normalize transport chain
    o_items = [(st, nt) for st in range(SQ // 128) for nt in range(2)]
    for idx, (st, nt) in enumerate(o_items[:10]):
        sched.setdefault((5, 2 + idx), []).append(
            lambda st=st, nt=nt: o_partial(st, nt))
    o_items_tail = o_items[10:]

    # ---- attention ----
    pending_norm = [None]
    for hp in range(DC):
        pvs = [pvps.tile([128, SQ], F32, tag="pv", name=f"pv{i}")
               for i in range(2)]
        ex = [[None] * ST, [None] * ST]

        def do_pv(j, hp=hp, pvs=pvs, ex=ex):
            for i in range(2):
                h = 2 * hp + i
                for jj in range(SQ // 512):
                    nc.tensor.matmul(
                        pvs[i][:, ts(jj, 512)],
                        Vaug[j][:, ds(h * 65, 128)],
                        ex[i][j][:, ts(jj, 512)],
                        start=(j == 0), stop=(j == ST - 1))

        for skt in range(ST):
            fns = list(sched.get((hp, skt), ()))
            for i in range(2):
                sc = scps.tile([128, SQ], F32, tag="sc", name="sc")
                for j in range(SQ // 512):
                    nc.tensor.matmul(
                        sc[:, ts(j, 512)],
                        KT[hp][ds(64 * i, 64), ts(skt, 128)],
                        QT[hp][ds(64 * i, 64), ts(j, 512)],
                        start=True, stop=True,
                        tile_position=(64 * i, 0))
                e_t = expp.tile([128, SQ], BF16, tag="exp", name="expt")
                nc.scalar.activation(e_t[:], sc[:], AF.Exp, scale=0.125)
                ex[i][skt] = e_t
                if fns:  # interleave one proj item after each head's scores
                    fns.pop(0)()
            if skt == 1 and pending_norm[0] is not None:
                pending_norm[0]()
                pending_norm[0] = None
            if skt >= LAG:
                do_pv(skt - LAG)
            for fn in fns:
                fn()
        for j in range(ST - LAG, ST):
            do_pv(j)

        # normalize: psum row 64 = softmax denominator (ones column in
        # Vaug). Transport (DMA to partition-major, fast [128,8] exact
        # reciprocal, DMA back, broadcast) emitted now; the psum-releasing
        # scale-evict muls are deferred into the next pair's step 1 so the
        # PE stream stays dense across the pair boundary.
        rbs_list = []
        for i in range(2):
            rsrow = normp.tile([1, SQ], F32, tag=f"rsrow_{i}", name="rsrow")
            for c in range(2):
                nc.vector.tensor_copy(rsrow[:, ts(c, 512)],
                                      pvs[i][ds(64, 1), ts(c, 512)])
            c8 = normp.tile([128, SQ // 128], F32, tag=f"c8_{i}", name="c8")
            nc.sync.dma_start(c8[:], rsrow[:])
            r8 = normp.tile([128, SQ // 128], F32, tag=f"r8_{i}", name="r8")
            nc.vector.reciprocal(r8[:], c8[:])
            recb = normp.tile([1, SQ], F32, tag=f"recb_{i}", name="recb")
            nc.sync.dma_start(recb[:], r8[:])
            rbs = normp.tile([64, SQ], F32, tag=f"rbs_{i}", name="rbs")
            nc.gpsimd.partition_broadcast(rbs[:], recb[:])
            rbs_list.append(rbs)

        def norm_pair(hp=hp, pvs=pvs, rbs_list=rbs_list):
            for c in range(2):      # chunked so DVE stays interruptible;
                for i in range(2):  # c-outer so o-tail st 0-3 unblock first
                    nc.vector.tensor_mul(
                        attnT[hp][ds(64 * i, 64), ts(c, 512)],
                        pvs[i][ds(0, 64), ts(c, 512)],
                        rbs_list[i][:, ts(c, 512)])
        pending_norm[0] = norm_pair
    for st, nt in o_items_tail:  # overlap the last pair's norm transport
        o_partial(st, nt)
    pending_norm[0]()
    att_ctx.close()

    # ---- o_proj tail: chunk-5 matmul + add to the pair-5 partials,
    # evict-adds split across ScalarE/VectorE, one [128, 768] DMA per tile ----
    with (tc.tile_pool(name="o_ps", bufs=4, space="PSUM") as ops,
          tc.tile_pool(name="o_sb", bufs=3) as osb):
        for st in range(SQ // 128):
            o = osb.tile([128, D], F32, tag="o_out", name="o_out")
            for nt in range(2):
                p = ops.tile([128, 384], F32, tag="o", name="o_p")
                nc.tensor.matmul(p[:], attnT[DC - 1][:, ts(st, 128)],
                                 wo_sb[DC - 1][:, ts(nt, 384)],
                                 start=True, stop=True)
                nc.vector.tensor_add(o[:, ts(nt, 384)], p[:],
                                     opart[st][:, ts(nt, 384)])
            nc.sync.dma_start(out[ts(st, 128), :], o[:])


_NC_CACHE = None


def _get_nc():
    global _NC_CACHE
    if _NC_CACHE is None:
        _NC_CACHE = build_nc()
    return _NC_CACHE


def _rope_tables(pos):
    # [128, n] cos/sin tables in T-layout: rows = 4 blocks of the 32
    # frequencies (2 heads x concat(freqs, freqs)); sin sign pattern folded
    # (rows 0:32 -> -sin for the -x2 half, rows 32:64 -> +sin, repeating)
    import ml_dtypes
    inv = ROPE_BASE ** (-np.arange(32, dtype=np.float64) / 32.0)
    ang = np.outer(inv, pos.astype(np.float64))  # [32, n]
    c32 = np.cos(ang)
    s32 = np.sin(ang)
    cosR = np.tile(c32, (4, 1)).astype(ml_dtypes.bfloat16)
    sinS = np.concatenate([-s32, s32, -s32, s32], axis=0).astype(
        ml_dtypes.bfloat16)
    return np.ascontiguousarray(cosR), np.ascontiguousarray(sinS)


def _pack_chunks(a):
    # [768, N] -> [128, 6*N] chunk-major (one flat DMA per tensor)
    n = a.shape[1]
    return np.ascontiguousarray(
        a.reshape(DC, 128, n).transpose(1, 0, 2).reshape(128, DC * n))


def _pack_emajor(a):
    # [768, 768] -> [128, 4608] output-chunk-major:
    # out[p, (e*DC+dc)*128+j] = a[dc*128+p, e*128+j]
    return np.ascontiguousarray(
        a.reshape(DC, 128, DC, 128).transpose(1, 2, 0, 3).reshape(128, -1))


def kernel(hidden_states, position_ids, wq, bq, wk, bk, wv, bv, wo,
           _trace=False):
    import ml_dtypes
    bf16 = ml_dtypes.bfloat16
    hidden_states = np.asarray(hidden_states, dtype=np.float32)
    position_ids = np.asarray(position_ids, dtype=np.int32)
    wqT = _pack_emajor(np.asarray(wq, np.float32).T.astype(bf16))
    wkT = _pack_emajor(np.asarray(wk, np.float32).T.astype(bf16))
    wvT = _pack_chunks(np.asarray(wv, np.float32).T.astype(bf16))
    woT = _pack_chunks(np.asarray(wo, np.float32).T.astype(bf16))
    bq_c = np.ascontiguousarray(np.asarray(bq, np.float32).reshape(DC, 128).T)
    bk_c = np.ascontiguousarray(np.asarray(bk, np.float32).reshape(DC, 128).T)
    bvB = np.ascontiguousarray(
        np.broadcast_to(np.asarray(bv, np.float32).astype(bf16)[None, :],
                        (128, D)))

    nc = _get_nc()
    in_maps = []
    for b in range(B):
        # key columns permuted per core so its own query half comes first
        hsT_b = hidden_states[b].T.astype(bf16)  # [768, 2048]
        cosK_b, sinK_b = _rope_tables(position_ids[b])
        for half in range(2):
            if half == 0:
                perm = lambda a: np.ascontiguousarray(a)
            else:
                perm = lambda a: np.ascontiguousarray(
                    np.concatenate([a[:, SQ:], a[:, :SQ]], axis=1))
            hs_p = perm(hsT_b).reshape(DC, 128, S)  # [dc, p, c] permuted
            # [own-half pack | other-half pack], each chunk-major
            hs_packed = np.concatenate(
                [hs_p[:, :, 0:SQ].transpose(1, 0, 2).reshape(128, DC * SQ),
                 hs_p[:, :, SQ:S].transpose(1, 0, 2).reshape(128, DC * SQ)],
                axis=1)
            in_maps.append({
                "hsP": np.ascontiguousarray(hs_packed),
                "cosK": perm(cosK_b), "sinK": perm(sinK_b),
                "wqT": wqT, "wkT": wkT, "wvT": wvT, "woT": woT,
                "bq": bq_c, "bk": bk_c, "bvB": bvB,
            })
    res = run_bass_kernel_spmd(nc, in_maps, list(range(N_CORES)),
                               trace=_trace)
    outp = np.empty((B, S, D), np.float32)
    for core in range(N_CORES):
        b, half = core // 2, core % 2
        outp[b, half * SQ:(half + 1) * SQ] = res.results[core]["out"]
    if _trace:
        kernel._last_exec_time_ns = res.exec_time_ns
        kernel._last_results = res
    return outp
